# revision 52
# baseline (speedup 1.0000x reference)
"""Trainium2 Bass kernel for single-head causal attention
(B=4, T=4096, E=1024, DH=128, fp32 in/out), sharded over 8 NeuronCores.

Sharding: 8 cores = 4 batches x 2 query-parity shards; the two cores of a
batch pair each compute K^T/V for only their own parity rows, then exchange
halves with an intra-pair AllGather through DRAM (bf16 payload).

All PE matmuls run in bf16 (fp32 PSUM accumulation): the x input arrives
pre-transposed from the host as bf16 x^T [E, T/2] (no on-chip x transposes),
projections and attention matmuls stream bf16 at ~1 col/cycle instead of
fp32's ~1.8. Diagonal-window score tiles only compute the causally live
column subrange; the single intra-tile triangular mask (own half) and the
parity-dependent partner-diagonal mask come in as a tiny [2,128,128] input,
keeping the program core-uniform. Softmax numerator (avt) and denominator
(den) accumulate in PSUM per half, are evacuated by the vector engine and
DMA'd out unnormalized; the host does the own+partner sum, the divide, and
the [dh,q] -> [q,dh] transpose during unsharding.
"""

import contextlib

import ml_dtypes
import numpy as np

import concourse.bass as bass
import concourse.mybir as mybir
import concourse.tile as tile
from concourse import bacc
from concourse.bass_utils import run_bass_kernel_spmd

P = 128
B, T, E, DH = 4, 4096, 1024, 128
ECH = E // P            # 8 e-chunks
NW = T // (8 * P)       # 4 windows of 8 key tiles
NG = NW                 # 4 attention groups of 512 queries per core
NKT = T // P            # 32 key tiles
QG = 4 * P              # 512 queries per group
T2 = T // 2             # 2048 rows per core
HW = NKT // 2           # 16 key tiles per half
NCORES = 8
SCALE = 1.0 / np.sqrt(DH)

f32 = mybir.dt.float32
bf16 = mybir.dt.bfloat16
i32 = mybir.dt.int32

BF16 = ml_dtypes.bfloat16


def _make_identity(nc, ident):
    nc.gpsimd.memset(ident, 0.0)
    nc.gpsimd.affine_select(
        out=ident, in_=ident,
        compare_op=mybir.AluOpType.not_equal,
        fill=1.0, base=0,
        pattern=[[-1, P]], channel_multiplier=1,
    )


def build_nc():
    nc = bacc.Bacc("TRN2", target_bir_lowering=False, debug=False,
                   num_devices=NCORES)
    xT = nc.dram_tensor("xt", [E, T2], bf16, kind="ExternalInput").ap()
    wq = nc.dram_tensor("wq", [E, DH], bf16, kind="ExternalInput").ap()
    wk = nc.dram_tensor("wk", [E, DH], bf16, kind="ExternalInput").ap()
    wv = nc.dram_tensor("wv", [E, DH], bf16, kind="ExternalInput").ap()
    masks = nc.dram_tensor("masks", [2, P, P], bf16, kind="ExternalInput").ap()
    psel = nc.dram_tensor("psel", [1, 1], i32, kind="ExternalInput").ap()
    out_num = nc.dram_tensor("out_num", [2 * NG, P, QG], f32,
                             kind="ExternalOutput").ap()
    out_den = nc.dram_tensor("out_den", [2 * NG, 1, QG], f32,
                             kind="ExternalOutput").ap()

    with tile.TileContext(nc) as tc:
        _emit(nc, tc, xT, wq, wk, wv, masks, psel, out_num, out_den)
    nc.compile()
    return nc


def _emit(nc, tc, xT, wq, wk, wv, masks, psel, out_num, out_den):
    ctx = contextlib.ExitStack()
    with ctx:
        const = ctx.enter_context(tc.tile_pool(name="const", bufs=1))
        xt_pool = ctx.enter_context(tc.tile_pool(name="xt", bufs=4))
        kv_pool = ctx.enter_context(tc.tile_pool(name="kv", bufs=1))
        vtt_pool = ctx.enter_context(tc.tile_pool(name="vtt", bufs=2))
        pt_pool = ctx.enter_context(tc.tile_pool(name="pt", bufs=6))
        osb_pool = ctx.enter_context(tc.tile_pool(name="osb", bufs=3))
        dsb_pool = ctx.enter_context(tc.tile_pool(name="dsb", bufs=3))
        dram_pool = ctx.enter_context(
            tc.tile_pool(name="ccd", bufs=2, space="DRAM"))
        scr_psum = ctx.enter_context(
            tc.tile_pool(name="scrp", bufs=1, space="PSUM"))
        st_psum = ctx.enter_context(
            tc.tile_pool(name="stp", bufs=4, space="PSUM"))
        avt_psum = ctx.enter_context(
            tc.tile_pool(name="avtp", bufs=2, space="PSUM"))
        den_psum = ctx.enter_context(
            tc.tile_pool(name="denp", bufs=1, space="PSUM"))

        # ---- first x^T window before everything else (scalar DGE ring,
        # so the x loads never queue behind the weight/mask loads on the
        # sync ring) ----
        xt0 = xt_pool.tile([P, ECH, QG], bf16, name="xt")
        for ec in range(ECH):
            nc.scalar.dma_start(xt0[:, ec, :], xT[ec * P:(ec + 1) * P, 0:QG])

        # ---- constants ----
        identf = const.tile([P, P], f32)
        _make_identity(nc, identf)
        ident = const.tile([P, P], bf16)
        nc.vector.tensor_copy(ident[:], identf[:])
        # full 128-col all-ones stationary: a 1-col stationary breaks the
        # PE's fast-weight-load overlap (~+160ns/pair measured); the full
        # bank output costs nothing since a [1,QG] PSUM tile pads to a
        # bank anyway. Every output row equals the denominator; the host
        # reads row 0.
        ones_b = const.tile([P, P], bf16)
        nc.gpsimd.memset(ones_b, 1.0)

        psel_sb = const.tile([1, 1], i32)
        nc.sync.dma_start(psel_sb[:], psel[:])
        par_idx = nc.values_load(psel_sb[0:1, 0:1], min_val=0, max_val=1,
                                 skip_runtime_bounds_check=True)

        # PE warmup + exp-table preload during initial DMA wait
        warm = st_psum.tile([P, P], f32, tag="st", name="warm")
        for _ in range(40):
            nc.tensor.matmul(warm[:], ident[:], ident[:],
                             start=True, stop=True)
        wpt = pt_pool.tile([P, P], bf16, tag="pt", name="wpt")
        nc.scalar.activation(wpt[:], warm[:],
                             mybir.ActivationFunctionType.Exp, scale=0.0)

        w_sb = {}
        for name, wap in (("wk", wk), ("wv", wv), ("wq", wq)):
            wt = const.tile([P, ECH, DH], bf16, name=f"{name}_sb")
            nc.sync.dma_start(
                wt[:], wap.rearrange("(eo ei) d -> ei eo d", ei=P))
            w_sb[name] = wt

        masks_sb = const.tile([P, 2, P], bf16)

        cc_outs = []

        kt_own = kv_pool.tile([P, HW * P], bf16)
        v_own = kv_pool.tile([P, HW * P], bf16)
        kt_par = kv_pool.tile([P, HW * P], bf16)
        v_par = kv_pool.tile([P, HW * P], bf16)
        qt_sb = kv_pool.tile([P, NG * QG], bf16)

        def prep(w):
            if w == 0:
                xt = xt0
            else:
                xt = xt_pool.tile([P, ECH, QG], bf16, name="xt")
                for ec in range(ECH):
                    nc.scalar.dma_start(
                        xt[:, ec, :],
                        xT[ec * P:(ec + 1) * P, w * QG:(w + 1) * QG])
            ws = slice(4 * w * P, (4 * w + 4) * P)
            cc_in = dram_pool.tile([P, 2 * QG], bf16, name="cc_in", bufs=4)

            ktp = scr_psum.tile([P, QG], f32, tag="scr", name="ktp")
            for ec in range(ECH):
                nc.tensor.matmul(ktp[:], w_sb["wk"][:, ec, :], xt[:, ec, :],
                                 start=(ec == 0), stop=(ec == ECH - 1))
            nc.vector.tensor_copy(kt_own[:, ws], ktp[:])
            nc.sync.dma_start(cc_in[:, :QG], kt_own[:, ws])

            vtp = scr_psum.tile([P, QG], f32, tag="scr", name="vtp")
            for ec in range(ECH):
                nc.tensor.matmul(vtp[:], w_sb["wv"][:, ec, :], xt[:, ec, :],
                                 start=(ec == 0), stop=(ec == ECH - 1))
            vtt = vtt_pool.tile([P, QG], bf16, name="vtt")
            nc.vector.tensor_copy(vtt[:], vtp[:])
            vnp = scr_psum.tile([P, QG], bf16, tag="scr", name="vnp")
            for kb in range(4):
                nc.tensor.transpose(
                    vnp[:, kb * P:(kb + 1) * P],
                    vtt[:, kb * P:(kb + 1) * P],
                    ident[:])
            nc.vector.tensor_copy(v_own[:, ws], vnp[:])
            nc.sync.dma_start(cc_in[:, QG:], v_own[:, ws])

            # exchange halves within the batch pair; the partner-block
            # distribution DMAs are issued later (dist), so no DMA that
            # waits on a collective ever sits ahead of the next window's
            # cc_in writes in the queues
            cc_out = dram_pool.tile([2, P, 2 * QG], bf16, name="cc_out",
                                    bufs=4)
            nc.gpsimd.collective_compute(
                "AllGather", mybir.AluOpType.bypass,
                replica_groups=[[0, 1], [2, 3], [4, 5], [6, 7]],
                ins=[cc_in[:]],
                outs=[cc_out[:]],
            )
            cc_outs.append(cc_out)

            qtp = scr_psum.tile([P, QG], f32, tag="scr", name="qtp")
            for ec in range(ECH):
                nc.tensor.matmul(qtp[:], w_sb["wq"][:, ec, :], xt[:, ec, :],
                                 start=(ec == 0), stop=(ec == ECH - 1))
            nc.vector.tensor_copy(qt_sb[:, QG * w:QG * (w + 1)], qtp[:])


        def attn_half(g, par):
            src_k = kt_par if par else kt_own
            src_v = v_par if par else v_own
            qt_g = qt_sb[:, QG * g:QG * (g + 1)]
            avt = avt_psum.tile([P, QG], f32, name="avt")
            den = den_psum.tile([P, QG], f32, name="den")
            tiles = [(w, s, w == g) for w in range(g + 1) for s in range(4)]
            n = len(tiles)
            pts = [None] * n

            # score matmul + exp for tile i (issued ahead of the avt/den of
            # earlier tiles so the PE never stalls on the scalar engine)
            def emit_st(i):
                w, s, diag = tiles[i]
                kc = (4 * w + s) * P
                cs = s * P if diag else 0
                st = st_psum.tile([P, QG], f32, tag="st", name="st")
                nc.tensor.matmul(st[:, cs:], src_k[:, kc:kc + P],
                                 qt_g[:, cs:], start=True, stop=True)
                pt = pt_pool.tile([P, QG], bf16, tag="pt", name="pt")
                nc.scalar.activation(pt[:, cs:], st[:, cs:],
                                     mybir.ActivationFunctionType.Exp,
                                     scale=SCALE)
                if diag:
                    nc.vector.tensor_mul(
                        pt[:, cs:cs + P], pt[:, cs:cs + P],
                        masks_sb[:, 1 if par else 0, :])
                pts[i] = (pt, cs)

            # paired emission: [st(i+2), st(i+3)] lookahead, then
            # avt(i), avt(i+1), den(i), den(i+1) — consecutive same-bank
            # matmuls avoid the ~43ns PSUM bank-switch penalty
            def emit_av(i):
                w, s, diag = tiles[i]
                kc = (4 * w + s) * P
                pt, cs = pts[i]
                nc.tensor.matmul(avt[:, cs:], src_v[:, kc:kc + P],
                                 pt[:, cs:], start=(i == 0),
                                 stop=(i == n - 1))

            def emit_den(i):
                pt, cs = pts[i]
                nc.tensor.matmul(den[:, cs:], ones_b[:], pt[:, cs:],
                                 start=(i == 0), stop=(i == n - 1))

            emit_st(0)
            emit_st(1)
            for i in range(0, n, 2):
                if i + 2 < n:
                    emit_st(i + 2)
                if i + 3 < n:
                    emit_st(i + 3)
                emit_av(i)
                emit_av(i + 1)
                if not tiles[i][2] and not tiles[i + 1][2]:
                    # full-width pair: pre-sum the two pt tiles on the
                    # vector engine and run a single den matmul
                    pt_i, _ = pts[i]
                    pt_j, _ = pts[i + 1]
                    psum_t = pt_pool.tile([P, QG], bf16, tag="pt",
                                          name="ptsum")
                    nc.vector.tensor_add(psum_t[:], pt_i[:], pt_j[:])
                    nc.tensor.matmul(den[:], ones_b[:], psum_t[:],
                                     start=(i == 0), stop=(i + 1 == n - 1))
                else:
                    emit_den(i)
                    emit_den(i + 1)
            h = (4 if par else 0) + g
            den_sb = dsb_pool.tile([1, QG], f32, name="den_sb")
            nc.vector.tensor_copy(den_sb[:], den[0:1, :])
            nc.sync.dma_start(out_den[h], den_sb[:])
            avt_sb = osb_pool.tile([P, QG], f32, name="avt_sb")
            nc.vector.tensor_copy(avt_sb[:], avt[:])
            nc.sync.dma_start(out_num[h], avt_sb[:])

        def dist(w):
            ws = slice(4 * w * P, (4 * w + 4) * P)
            nc.sync.dma_start(kt_par[:, ws],
                              cc_outs[w][bass.ds(par_idx, 1), :, 0:QG])
            nc.sync.dma_start(v_par[:, ws],
                              cc_outs[w][bass.ds(par_idx, 1), :, QG:2 * QG])

        prep(0)
        nc.sync.dma_start(masks_sb[:], masks.rearrange("j p c -> p j c"))
        attn_half(0, False)
        prep(1)
        prep(2)
        attn_half(1, False)
        prep(3)
        attn_half(2, False)
        dist(0)
        dist(1)
        attn_half(3, False)
        dist(2)
        dist(3)
        for g in range(NG):
            attn_half(g, True)


# ---------------- host side ----------------

def _own_tiles(p):
    return np.array([8 * w + p + 2 * a for w in range(NW) for a in range(4)])


def _masks(p):
    """[0]: intra-tile triangular causal mask (key row <= query col) for the
    own-half diagonal subtile; [1]: partner-half diagonal subtile mask —
    all-ones for p=1 (partner keys strictly older), all-zeros for p=0
    (partner keys strictly newer)."""
    m = np.zeros((2, P, P), np.float32)
    kl = np.arange(P)[:, None]
    ql = np.arange(P)[None, :]
    m[0][kl <= ql] = 1.0
    if p == 1:
        m[1][:] = 1.0
    return m.astype(BF16)


_NC_CACHE = []


def _get_nc():
    if not _NC_CACHE:
        _NC_CACHE.append(build_nc())
    return _NC_CACHE[0]


def _run(norm_inputs, Wq, Wk, Wv, **spmd_kwargs):
    nc = _get_nc()
    xf = np.asarray(norm_inputs, np.float32)
    wqb = np.asarray(Wq, np.float32).astype(BF16)
    wkb = np.asarray(Wk, np.float32).astype(BF16)
    wvb = np.asarray(Wv, np.float32).astype(BF16)
    in_maps = []
    for c in range(NCORES):
        b, p = c // 2, c % 2
        xp = xf[b].reshape(NKT, P, E)[_own_tiles(p)].reshape(T2, E)
        xTc = np.ascontiguousarray(xp.T.astype(BF16))
        in_maps.append({
            "xt": xTc, "wq": wqb, "wk": wkb, "wv": wvb,
            "masks": _masks(p),
            "psel": np.array([[1 - p]], np.int32),
        })
    res = run_bass_kernel_spmd(nc, in_maps, core_ids=list(range(NCORES)),
                               **spmd_kwargs)
    outf = np.empty((B, T, DH), np.float32)
    for c in range(NCORES):
        b, p = c // 2, c % 2
        num = res.results[c]["out_num"]          # [8, P, QG] dh-major
        den = res.results[c]["out_den"]          # [8, 1, QG]
        tot = num[:NG] + num[NG:]                # [NG, P(dh), QG]
        dsum = den[:NG] + den[NG:]               # [NG, 1, QG]
        o = tot / dsum                           # [NG, dh, q]
        o = o.reshape(NG, P, 4, P).transpose(0, 2, 3, 1)   # [g, a, r, dh]
        full = outf[b].reshape(NKT, P, DH)
        for i in range(NG):
            for a in range(4):
                full[8 * i + p + 2 * a] = o[i, a]
    return outf, res


def kernel(norm_inputs, Wq, Wk, Wv):
    outf, _ = _run(norm_inputs, Wq, Wk, Wv)
    return outf


# revision 53
# speedup vs baseline: 1.1545x; 1.1545x over previous
"""Trainium2 Bass kernel for single-head causal attention
(B=4, T=4096, E=1024, DH=128, fp32 in/out), sharded over 8 NeuronCores.

Sharding: 8 cores = 4 batches x 2 query-parity shards; the two cores of a
batch pair each compute K^T/V for only their own parity rows, then exchange
halves with an intra-pair AllGather through DRAM (bf16 payload).

All PE matmuls run in bf16 (fp32 PSUM accumulation): the x input arrives
pre-transposed from the host as bf16 x^T [E, T/2] (no on-chip x transposes),
projections and attention matmuls stream bf16 at ~1 col/cycle instead of
fp32's ~1.8. Diagonal-window score tiles only compute the causally live
column subrange; the single intra-tile triangular mask (own half) and the
parity-dependent partner-diagonal mask come in as a tiny [2,128,128] input,
keeping the program core-uniform. Softmax numerator (avt) and denominator
(den) accumulate in PSUM per half, are evacuated by the vector engine and
DMA'd out unnormalized; the host does the own+partner sum, the divide, and
the [dh,q] -> [q,dh] transpose during unsharding.
"""

import contextlib

import ml_dtypes
import numpy as np

import concourse.bass as bass
import concourse.mybir as mybir
import concourse.tile as tile
from concourse import bacc
from concourse.bass_utils import run_bass_kernel_spmd

P = 128
B, T, E, DH = 4, 4096, 1024, 128
ECH = E // P            # 8 e-chunks
NW = T // (8 * P)       # 4 windows of 8 key tiles
NG = NW                 # 4 attention groups of 512 queries per core
NKT = T // P            # 32 key tiles
QG = 4 * P              # 512 queries per group
T2 = T // 2             # 2048 rows per core
HW = NKT // 2           # 16 key tiles per half
NCORES = 8
SCALE = 1.0 / np.sqrt(DH)

f32 = mybir.dt.float32
bf16 = mybir.dt.bfloat16
i32 = mybir.dt.int32

BF16 = ml_dtypes.bfloat16


def _make_identity(nc, ident):
    nc.gpsimd.memset(ident, 0.0)
    nc.gpsimd.affine_select(
        out=ident, in_=ident,
        compare_op=mybir.AluOpType.not_equal,
        fill=1.0, base=0,
        pattern=[[-1, P]], channel_multiplier=1,
    )


def build_nc():
    nc = bacc.Bacc("TRN2", target_bir_lowering=False, debug=False,
                   num_devices=NCORES)
    xT = nc.dram_tensor("xt", [E, T2], bf16, kind="ExternalInput").ap()
    wq = nc.dram_tensor("wq", [P, ECH, DH], bf16, kind="ExternalInput").ap()
    wk = nc.dram_tensor("wk", [P, ECH, DH], bf16, kind="ExternalInput").ap()
    wv = nc.dram_tensor("wv", [P, ECH, DH], bf16, kind="ExternalInput").ap()
    masks = nc.dram_tensor("masks", [P, 2, P], bf16, kind="ExternalInput").ap()
    psel = nc.dram_tensor("psel", [1, 1], i32, kind="ExternalInput").ap()
    out_num = nc.dram_tensor("out_num", [2 * NG, P, QG], f32,
                             kind="ExternalOutput").ap()
    out_den = nc.dram_tensor("out_den", [2 * NG, 1, QG], f32,
                             kind="ExternalOutput").ap()

    with tile.TileContext(nc) as tc:
        _emit(nc, tc, xT, wq, wk, wv, masks, psel, out_num, out_den)
    nc.compile()
    return nc


def _emit(nc, tc, xT, wq, wk, wv, masks, psel, out_num, out_den):
    ctx = contextlib.ExitStack()
    with ctx:
        const = ctx.enter_context(tc.tile_pool(name="const", bufs=1))
        xt_pool = ctx.enter_context(tc.tile_pool(name="xt", bufs=4))
        kv_pool = ctx.enter_context(tc.tile_pool(name="kv", bufs=1))
        vtt_pool = ctx.enter_context(tc.tile_pool(name="vtt", bufs=2))
        pt_pool = ctx.enter_context(tc.tile_pool(name="pt", bufs=6))
        osb_pool = ctx.enter_context(tc.tile_pool(name="osb", bufs=3))
        dsb_pool = ctx.enter_context(tc.tile_pool(name="dsb", bufs=3))
        dram_pool = ctx.enter_context(
            tc.tile_pool(name="ccd", bufs=2, space="DRAM"))
        scr_psum = ctx.enter_context(
            tc.tile_pool(name="scrp", bufs=1, space="PSUM"))
        st_psum = ctx.enter_context(
            tc.tile_pool(name="stp", bufs=4, space="PSUM"))
        avt_psum = ctx.enter_context(
            tc.tile_pool(name="avtp", bufs=2, space="PSUM"))
        den_psum = ctx.enter_context(
            tc.tile_pool(name="denp", bufs=1, space="PSUM"))

        # ---- first x^T window before everything else (scalar DGE ring,
        # so the x loads never queue behind the weight/mask loads on the
        # sync ring) ----
        xt0 = xt_pool.tile([P, ECH, QG], bf16, name="xt")
        for ec in range(ECH):
            nc.scalar.dma_start(xt0[:, ec, :], xT[ec * P:(ec + 1) * P, 0:QG])

        # ---- constants ----
        identf = const.tile([P, P], f32)
        _make_identity(nc, identf)
        ident = const.tile([P, P], bf16)
        nc.vector.tensor_copy(ident[:], identf[:])
        # full 128-col all-ones stationary: a 1-col stationary breaks the
        # PE's fast-weight-load overlap (~+160ns/pair measured); the full
        # bank output costs nothing since a [1,QG] PSUM tile pads to a
        # bank anyway. Every output row equals the denominator; the host
        # reads row 0.
        ones_b = const.tile([P, P], bf16)
        nc.gpsimd.memset(ones_b, 1.0)

        psel_sb = const.tile([1, 1], i32)
        nc.sync.dma_start(psel_sb[:], psel[:])
        par_idx = nc.values_load(psel_sb[0:1, 0:1], min_val=0, max_val=1,
                                 skip_runtime_bounds_check=True)

        # PE warmup + exp-table preload during initial DMA wait
        warm = st_psum.tile([P, P], f32, tag="st", name="warm")
        for _ in range(40):
            nc.tensor.matmul(warm[:], ident[:], ident[:],
                             start=True, stop=True)
        wpt = pt_pool.tile([P, P], bf16, tag="pt", name="wpt")
        nc.scalar.activation(wpt[:], warm[:],
                             mybir.ActivationFunctionType.Exp, scale=0.0)

        w_sb = {}
        for name, wap in (("wk", wk), ("wv", wv), ("wq", wq)):
            wt = const.tile([P, ECH, DH], bf16, name=f"{name}_sb")
            nc.sync.dma_start(wt[:], wap[:])
            w_sb[name] = wt

        masks_sb = const.tile([P, 2, P], bf16)

        cc_outs = []

        kt_own = kv_pool.tile([P, HW * P], bf16)
        v_own = kv_pool.tile([P, HW * P], bf16)
        kt_par = kv_pool.tile([P, HW * P], bf16)
        v_par = kv_pool.tile([P, HW * P], bf16)
        qt_sb = kv_pool.tile([P, NG * QG], bf16)

        def prep(w):
            if w == 0:
                xt = xt0
            else:
                xt = xt_pool.tile([P, ECH, QG], bf16, name="xt")
                for ec in range(ECH):
                    nc.scalar.dma_start(
                        xt[:, ec, :],
                        xT[ec * P:(ec + 1) * P, w * QG:(w + 1) * QG])
            ws = slice(4 * w * P, (4 * w + 4) * P)
            cc_in = dram_pool.tile([P, 2 * QG], bf16, name="cc_in", bufs=4)

            ktp = scr_psum.tile([P, QG], f32, tag="scr", name="ktp")
            for ec in range(ECH):
                nc.tensor.matmul(ktp[:], w_sb["wk"][:, ec, :], xt[:, ec, :],
                                 start=(ec == 0), stop=(ec == ECH - 1))
            nc.vector.tensor_copy(kt_own[:, ws], ktp[:])
            nc.sync.dma_start(cc_in[:, :QG], kt_own[:, ws])

            vtp = scr_psum.tile([P, QG], f32, tag="scr", name="vtp")
            for ec in range(ECH):
                nc.tensor.matmul(vtp[:], w_sb["wv"][:, ec, :], xt[:, ec, :],
                                 start=(ec == 0), stop=(ec == ECH - 1))
            vtt = vtt_pool.tile([P, QG], bf16, name="vtt")
            nc.vector.tensor_copy(vtt[:], vtp[:])
            vnp = scr_psum.tile([P, QG], bf16, tag="scr", name="vnp")
            for kb in range(4):
                nc.tensor.transpose(
                    vnp[:, kb * P:(kb + 1) * P],
                    vtt[:, kb * P:(kb + 1) * P],
                    ident[:])
            nc.vector.tensor_copy(v_own[:, ws], vnp[:])
            nc.sync.dma_start(cc_in[:, QG:], v_own[:, ws])

            # exchange halves within the batch pair; the partner-block
            # distribution DMAs are issued later (dist), so no DMA that
            # waits on a collective ever sits ahead of the next window's
            # cc_in writes in the queues
            cc_out = dram_pool.tile([2, P, 2 * QG], bf16, name="cc_out",
                                    bufs=4)
            nc.gpsimd.collective_compute(
                "AllGather", mybir.AluOpType.bypass,
                replica_groups=[[0, 1], [2, 3], [4, 5], [6, 7]],
                ins=[cc_in[:]],
                outs=[cc_out[:]],
            )
            cc_outs.append(cc_out)

            qtp = scr_psum.tile([P, QG], f32, tag="scr", name="qtp")
            for ec in range(ECH):
                nc.tensor.matmul(qtp[:], w_sb["wq"][:, ec, :], xt[:, ec, :],
                                 start=(ec == 0), stop=(ec == ECH - 1))
            nc.vector.tensor_copy(qt_sb[:, QG * w:QG * (w + 1)], qtp[:])


        def attn_half(g, par):
            src_k = kt_par if par else kt_own
            src_v = v_par if par else v_own
            qt_g = qt_sb[:, QG * g:QG * (g + 1)]
            avt = avt_psum.tile([P, QG], f32, name="avt")
            den = den_psum.tile([P, QG], f32, name="den")
            tiles = [(w, s, w == g) for w in range(g + 1) for s in range(4)]
            n = len(tiles)
            pts = [None] * n

            # score matmul + exp for tile i (issued ahead of the avt/den of
            # earlier tiles so the PE never stalls on the scalar engine)
            def emit_st(i):
                w, s, diag = tiles[i]
                kc = (4 * w + s) * P
                cs = s * P if diag else 0
                st = st_psum.tile([P, QG], f32, tag="st", name="st")
                nc.tensor.matmul(st[:, cs:], src_k[:, kc:kc + P],
                                 qt_g[:, cs:], start=True, stop=True)
                pt = pt_pool.tile([P, QG], bf16, tag="pt", name="pt")
                nc.scalar.activation(pt[:, cs:], st[:, cs:],
                                     mybir.ActivationFunctionType.Exp,
                                     scale=SCALE)
                if diag:
                    nc.vector.tensor_mul(
                        pt[:, cs:cs + P], pt[:, cs:cs + P],
                        masks_sb[:, 1 if par else 0, :])
                pts[i] = (pt, cs)

            # paired emission: [st(i+2), st(i+3)] lookahead, then
            # avt(i), avt(i+1), den(i), den(i+1) — consecutive same-bank
            # matmuls avoid the ~43ns PSUM bank-switch penalty
            def emit_av(i):
                w, s, diag = tiles[i]
                kc = (4 * w + s) * P
                pt, cs = pts[i]
                nc.tensor.matmul(avt[:, cs:], src_v[:, kc:kc + P],
                                 pt[:, cs:], start=(i == 0),
                                 stop=(i == n - 1))

            def emit_den(i):
                pt, cs = pts[i]
                nc.tensor.matmul(den[:, cs:], ones_b[:], pt[:, cs:],
                                 start=(i == 0), stop=(i == n - 1))

            emit_st(0)
            emit_st(1)
            for i in range(0, n, 2):
                if i + 2 < n:
                    emit_st(i + 2)
                if i + 3 < n:
                    emit_st(i + 3)
                emit_av(i)
                emit_av(i + 1)
                if not tiles[i][2] and not tiles[i + 1][2]:
                    # full-width pair: pre-sum the two pt tiles on the
                    # vector engine and run a single den matmul
                    pt_i, _ = pts[i]
                    pt_j, _ = pts[i + 1]
                    psum_t = pt_pool.tile([P, QG], bf16, tag="pt",
                                          name="ptsum")
                    nc.vector.tensor_add(psum_t[:], pt_i[:], pt_j[:])
                    nc.tensor.matmul(den[:], ones_b[:], psum_t[:],
                                     start=(i == 0), stop=(i + 1 == n - 1))
                else:
                    emit_den(i)
                    emit_den(i + 1)
            h = (4 if par else 0) + g
            den_sb = dsb_pool.tile([1, QG], f32, name="den_sb")
            nc.vector.tensor_copy(den_sb[:], den[0:1, :])
            nc.sync.dma_start(out_den[h], den_sb[:])
            avt_sb = osb_pool.tile([P, QG], f32, name="avt_sb")
            nc.vector.tensor_copy(avt_sb[:], avt[:])
            nc.sync.dma_start(out_num[h], avt_sb[:])

        def dist(w):
            ws = slice(4 * w * P, (4 * w + 4) * P)
            nc.sync.dma_start(kt_par[:, ws],
                              cc_outs[w][bass.ds(par_idx, 1), :, 0:QG])
            nc.sync.dma_start(v_par[:, ws],
                              cc_outs[w][bass.ds(par_idx, 1), :, QG:2 * QG])

        prep(0)
        nc.sync.dma_start(masks_sb[:], masks[:])
        attn_half(0, False)
        prep(1)
        prep(2)
        attn_half(1, False)
        prep(3)
        attn_half(2, False)
        dist(0)
        dist(1)
        attn_half(3, False)
        dist(2)
        dist(3)
        for g in range(NG):
            attn_half(g, True)


# ---------------- host side ----------------

def _own_tiles(p):
    return np.array([8 * w + p + 2 * a for w in range(NW) for a in range(4)])


def _masks(p):
    """[0]: intra-tile triangular causal mask (key row <= query col) for the
    own-half diagonal subtile; [1]: partner-half diagonal subtile mask —
    all-ones for p=1 (partner keys strictly older), all-zeros for p=0
    (partner keys strictly newer)."""
    m = np.zeros((2, P, P), np.float32)
    kl = np.arange(P)[:, None]
    ql = np.arange(P)[None, :]
    m[0][kl <= ql] = 1.0
    if p == 1:
        m[1][:] = 1.0
    return np.ascontiguousarray(m.transpose(1, 0, 2)).astype(BF16)


_NC_CACHE = []


def _get_nc():
    if not _NC_CACHE:
        _NC_CACHE.append(build_nc())
    return _NC_CACHE[0]


def _run(norm_inputs, Wq, Wk, Wv, **spmd_kwargs):
    nc = _get_nc()
    xf = np.asarray(norm_inputs, np.float32)
    def _wlay(w):
        wf = np.asarray(w, np.float32).astype(BF16)
        return np.ascontiguousarray(
            wf.reshape(ECH, P, DH).transpose(1, 0, 2))
    wqb = _wlay(Wq)
    wkb = _wlay(Wk)
    wvb = _wlay(Wv)
    in_maps = []
    for c in range(NCORES):
        b, p = c // 2, c % 2
        xp = xf[b].reshape(NKT, P, E)[_own_tiles(p)].reshape(T2, E)
        xTc = np.ascontiguousarray(xp.T.astype(BF16))
        in_maps.append({
            "xt": xTc, "wq": wqb, "wk": wkb, "wv": wvb,
            "masks": _masks(p),
            "psel": np.array([[1 - p]], np.int32),
        })
    res = run_bass_kernel_spmd(nc, in_maps, core_ids=list(range(NCORES)),
                               **spmd_kwargs)
    outf = np.empty((B, T, DH), np.float32)
    for c in range(NCORES):
        b, p = c // 2, c % 2
        num = res.results[c]["out_num"]          # [8, P, QG] dh-major
        den = res.results[c]["out_den"]          # [8, 1, QG]
        tot = num[:NG] + num[NG:]                # [NG, P(dh), QG]
        dsum = den[:NG] + den[NG:]               # [NG, 1, QG]
        o = tot / dsum                           # [NG, dh, q]
        o = o.reshape(NG, P, 4, P).transpose(0, 2, 3, 1)   # [g, a, r, dh]
        full = outf[b].reshape(NKT, P, DH)
        for i in range(NG):
            for a in range(4):
                full[8 * i + p + 2 * a] = o[i, a]
    return outf, res


def kernel(norm_inputs, Wq, Wk, Wv):
    outf, _ = _run(norm_inputs, Wq, Wk, Wv)
    return outf
